# revision 22
# baseline (speedup 1.0000x reference)
"""LIF spike kernel for Trainium2 (Bass/Tile), data-parallel over 8 NeuronCores.

Problem: x [32, 8, 128, 32, 32] fp32 -> spikes [32, 8, 128, 32, 32] fp32
    mem_t = mem_{t-1} * 0.25 + x_t ; spike = (mem >= 0.5) ; mem *= (1 - spike)

Sharding: batch dim (32) split 4-per-core across 8 cores; no cross-core comm.

v3 design (variant "pack", default):
  * All recurrence math in fp16 with DVE ops that have fast perf modes
    (scalar_tensor_tensor has NONE and costs 4.38us on [128,4096] fp32 or
    fp16; tensor_scalar fp16 runs 4x ~1.35us, tensor_tensor fp16 2x ~2.4us):
        u_t = q_{t-1} + x_t            tensor_tensor add   (2x)
        m_t = (u_t < 0.5) * 0.25       tensor_scalar       (4x)  in {0,0.25}
        q_t = m_t * u_t                tensor_tensor mult  (2x)  = tau*reset
  * Spike OUTPUT is bit-packed by the otherwise-idle PE: m_t = 0.25*(1-y_t),
    so PSUM += (2^(t+2) I) @ m_t accumulated over t gives 255 - packed_bits.
    One ACT copy (scale=-1, bias=255) casts PSUM -> uint8; host unpacks bits.
    This removes the per-step ACT Sign op AND cuts store traffic 8x.
  * Host pre-transposes x to [T, C, BPC*HW] fp16 per core so every per-step
    load is one contiguous 1 MiB DMA (128 partitions x 8 KiB lines).
  * Loads ride the SP HWDGE ring; the single store rides the ACT ring.

fp16 rounding (one rounding per step, on u) flips ~3k of 33.5M spikes:
rel err ~1.8e-2 < 2e-2, deterministic for the fixed test seed.
"""

import os
import numpy as np

B, T, C, H, W = 32, 8, 128, 32, 32
HW = H * W
N_CORES = 8
BPC = B // N_CORES  # batches per core
FREE = BPC * HW  # 4096 free columns per core
TAU = 0.25
THRESH = 0.5
BANK = 512  # fp32 words per PSUM bank

_nc_cache = {}
LAST_RESULTS = None


def build_pack(loop_n=1, head_ch=4, t1_ch=2, tail_ch=2, dve_copies=True):
    import concourse.bacc as bacc
    import concourse.mybir as mybir
    from concourse.tile import TileContext

    f16 = mybir.dt.float16
    f32 = mybir.dt.float32
    u8 = mybir.dt.uint8
    Alu = mybir.AluOpType
    Act = mybir.ActivationFunctionType
    free = FREE

    nc = bacc.Bacc("TRN2", target_bir_lowering=False)
    x = nc.dram_tensor("x", [T, C, free], f16, kind="ExternalInput")
    y = nc.dram_tensor("y", [C, free], u8, kind="ExternalOutput")
    # stationary pack weights: W_t = 2^(t+2) * I so that W_t @ m_t sums to
    # 255 - packed spike bits over the 8 steps. Stored partition-major
    # [C, T, C] so the SBUF tile needs no rearrange.
    ws = np.zeros((C, T, C), dtype=np.float16)
    for t in range(T):
        ws[np.arange(C), t, np.arange(C)] = np.float16(2.0 ** (t + 2))
    w_d = nc.inline_tensor(ws, "w")

    with TileContext(nc) as tc:
        with (
            tc.tile_pool(name="xp", bufs=1) as xp,
            tc.tile_pool(name="spool", bufs=1) as spool,
            tc.tile_pool(name="yp", bufs=1) as yp,
            tc.tile_pool(name="cp", bufs=1) as cp,
            tc.tile_pool(name="ps", bufs=1, space="PSUM") as ps,
        ):
            xts = [xp.tile([C, free], f16, name=f"x{t}") for t in range(T)]
            us = [spool.tile([C, free], f16, name=f"u{i}") for i in range(2)]
            ms = [spool.tile([C, free], f16, name=f"m{i}") for i in range(2)]
            qs = [spool.tile([C, free], f16, name=f"q{i}") for i in range(2)]
            yt = yp.tile([C, free], u8, name="yt")
            wt = cp.tile([C, T, C], f16, name="wt")
            pt = ps.tile([C, free], f32, name="pt")
            nc.sync.dma_start(wt[:], w_d[:])

            nbank = free // BANK

            def body():
                # head: the first load lands in 4 column chunks so compute
                # starts after ~1/4 of it; remaining loads are whole tiles,
                # all queued ahead of stores on the SP ring
                hc = free // head_ch
                for j in range(head_ch):
                    sl = slice(j * hc, (j + 1) * hc)
                    nc.sync.dma_start(xts[0][:, sl], x[0][:, sl])
                for j in range(t1_ch):
                    sl = slice(j * (free // t1_ch), (j + 1) * (free // t1_ch))
                    nc.sync.dma_start(xts[1][:, sl], x[1][:, sl])
                for t in range(2, T):
                    nc.sync.dma_start(xts[t][:], x[t])
                q = None
                for t in range(T):
                    u = xts[0] if t == 0 else us[t % 2]
                    last = t == T - 1
                    m = ms[t % 2]
                    # head (t=0,1) and tail (t=7) run column-chunked so the
                    # pipeline fills/drains incrementally
                    if t == 0:
                        nch = head_ch
                    elif last:
                        nch = tail_ch
                    elif t == 1:
                        nch = t1_ch
                    else:
                        nch = 1
                    cw = free // nch
                    for jc in range(nch):
                        sl = slice(jc * cw, (jc + 1) * cw)
                        if t > 0:
                            nc.vector.tensor_tensor(
                                u[:, sl], q[:, sl], xts[t][:, sl], Alu.add
                            )
                        nc.vector.tensor_scalar(
                            m[:, sl], u[:, sl], THRESH, TAU, Alu.is_lt, Alu.mult
                        )
                        if not last:
                            nc.vector.tensor_tensor(
                                qs[t % 2][:, sl], m[:, sl], u[:, sl], Alu.mult
                            )
                        # pack: PSUM bank accumulates 2^(t+2) * m_t
                        for j in range(jc * cw // BANK, (jc + 1) * cw // BANK):
                            bs = slice(j * BANK, (j + 1) * BANK)
                            nc.tensor.matmul(
                                pt[:, bs], wt[:, t, :], m[:, bs],
                                start=(t == 0), stop=last,
                            )
                            if last:
                                # evacuate banks as they finish; DVE is idle
                                # after its last m-chunk, so split the
                                # PSUM->u8 copies between ACT and DVE
                                if j % 2 == 0 or not dve_copies:
                                    nc.scalar.activation(
                                        yt[:, bs], pt[:, bs], Act.Copy,
                                        bias=255.0, scale=-1.0,
                                    )
                                else:
                                    nc.vector.tensor_scalar(
                                        yt[:, bs], pt[:, bs], -1.0, 255.0,
                                        Alu.mult, Alu.add,
                                    )
                        if last:
                            nc.scalar.dma_start(y[:, sl], yt[:, sl])
                    if not last:
                        q = qs[t % 2]

            if loop_n > 1:
                with tc.For_i(0, loop_n):
                    body()
            else:
                body()
    nc.compile()
    return nc


def build_variant(variant, loop_n=1):
    if variant == "packa":
        # experimental accumulating-load variant (test-only module)
        from kernel_accum import build_packa

        return build_packa(loop_n=loop_n, nchunk=4)
    return build_pack(loop_n=loop_n)


def _get_nc():
    key = os.environ.get("LIF_VARIANT", "pack")
    if key not in _nc_cache:
        _nc_cache[key] = build_variant(key)
    return _nc_cache[key]


def _set_default_variant(v):
    os.environ["LIF_VARIANT"] = v


def host_prep(x):
    """x [B,T,C,H,W] fp32 -> list of per-core [T, C, BPC*HW] fp16 arrays."""
    xs = x.reshape(B, T, C, HW).astype(np.float16)
    return [
        np.ascontiguousarray(
            xs[i * BPC : (i + 1) * BPC].transpose(1, 2, 0, 3).reshape(T, C, FREE)
        )
        for i in range(N_CORES)
    ]


def host_decode(res_list):
    """Per-core packed bytes [C, FREE] -> full fp32 spikes [B,T,C,H,W]."""
    out = np.empty((B, T, C, HW), dtype=np.float32)
    for i, yi in enumerate(res_list):
        bits = np.unpackbits(
            yi.reshape(C, BPC, HW, 1), axis=-1, bitorder="little"
        )  # [C, BPC, HW, 8]
        out[i * BPC : (i + 1) * BPC] = bits.transpose(1, 3, 0, 2)
    return out.reshape(B, T, C, H, W)


def kernel(x):
    global LAST_RESULTS
    from concourse import bass_utils

    assert x.shape == (B, T, C, H, W) and x.dtype == np.float32
    nc = _get_nc()
    in_maps = [{"x": xi} for xi in host_prep(x)]
    res = bass_utils.run_bass_kernel_spmd(
        nc,
        in_maps,
        core_ids=list(range(N_CORES)),
        trace=bool(int(os.environ.get("LIF_TRACE", "0"))),
    )
    LAST_RESULTS = res
    return host_decode([res.results[i]["y"] for i in range(N_CORES)])


# revision 25
# speedup vs baseline: 1.0070x; 1.0070x over previous
"""LIF spike kernel for Trainium2 (Bass/Tile), data-parallel over 8 NeuronCores.

Problem: x [32, 8, 128, 32, 32] fp32 -> spikes [32, 8, 128, 32, 32] fp32
    mem_t = mem_{t-1} * 0.25 + x_t ; spike = (mem >= 0.5) ; mem *= (1 - spike)

Sharding: batch dim (32) split 4-per-core across 8 cores; no cross-core comm.

v3 design (variant "pack", default):
  * All recurrence math in fp16 with DVE ops that have fast perf modes
    (scalar_tensor_tensor has NONE and costs 4.38us on [128,4096] fp32 or
    fp16; tensor_scalar fp16 runs 4x ~1.35us, tensor_tensor fp16 2x ~2.4us):
        u_t = q_{t-1} + x_t            tensor_tensor add   (2x)
        m_t = (u_t < 0.5) * 0.25       tensor_scalar       (4x)  in {0,0.25}
        q_t = m_t * u_t                tensor_tensor mult  (2x)  = tau*reset
  * Spike OUTPUT is bit-packed by the otherwise-idle PE: m_t = 0.25*(1-y_t),
    so PSUM += (2^(t+2) I) @ m_t accumulated over t gives 255 - packed_bits.
    One ACT copy (scale=-1, bias=255) casts PSUM -> uint8; host unpacks bits.
    This removes the per-step ACT Sign op AND cuts store traffic 8x.
  * Host pre-transposes x to [T, C, BPC*HW] fp16 per core so every per-step
    load is one contiguous 1 MiB DMA (128 partitions x 8 KiB lines).
  * Loads ride the SP HWDGE ring; the single store rides the ACT ring.

fp16 rounding (one rounding per step, on u) flips 1772 of 33.5M spikes:
rel err 1.37e-2 < 2e-2, deterministic for the fixed test seed (matches a
numpy emulation of the device op order exactly).

Measured per-iteration (hardware-loop slope, single core): ~52 us, vs ~67 us
for the best scalar_tensor_tensor fp32 formulation. Rejected experiments:
GPSIMD column offload (gpsimd tensor_scalar is 15 ns/col; DVE 2-port ops
block the shared SBUF port), SWDGE accumulate-on-load (CCE fp16 add is
bit-identical, but Q7 descriptor generation contends with DVE 2-port ops:
61 us measured vs 45 us simulated).
"""

import os
import numpy as np

B, T, C, H, W = 32, 8, 128, 32, 32
HW = H * W
N_CORES = 8
BPC = B // N_CORES  # batches per core
FREE = BPC * HW  # 4096 free columns per core
TAU = 0.25
THRESH = 0.5
BANK = 512  # fp32 words per PSUM bank

_nc_cache = {}
LAST_RESULTS = None


def build_pack(loop_n=1, head_ch=4, t1_ch=2, tail_ch=2, dve_copies=True):
    import concourse.bacc as bacc
    import concourse.mybir as mybir
    from concourse.tile import TileContext

    f16 = mybir.dt.float16
    f32 = mybir.dt.float32
    u8 = mybir.dt.uint8
    Alu = mybir.AluOpType
    Act = mybir.ActivationFunctionType
    free = FREE

    nc = bacc.Bacc("TRN2", target_bir_lowering=False)
    x = nc.dram_tensor("x", [T, C, free], f16, kind="ExternalInput")
    y = nc.dram_tensor("y", [C, free], u8, kind="ExternalOutput")
    # stationary pack weights: W_t = 2^(t+2) * I so that W_t @ m_t sums to
    # 255 - packed spike bits over the 8 steps. Stored partition-major
    # [C, T, C] so the SBUF tile needs no rearrange.
    ws = np.zeros((C, T, C), dtype=np.float16)
    for t in range(T):
        ws[np.arange(C), t, np.arange(C)] = np.float16(2.0 ** (t + 2))
    w_d = nc.inline_tensor(ws, "w")

    with TileContext(nc) as tc:
        with (
            tc.tile_pool(name="xp", bufs=1) as xp,
            tc.tile_pool(name="spool", bufs=1) as spool,
            tc.tile_pool(name="yp", bufs=1) as yp,
            tc.tile_pool(name="cp", bufs=1) as cp,
            tc.tile_pool(name="ps", bufs=1, space="PSUM") as ps,
        ):
            xts = [xp.tile([C, free], f16, name=f"x{t}") for t in range(T)]
            us = [spool.tile([C, free], f16, name=f"u{i}") for i in range(2)]
            ms = [spool.tile([C, free], f16, name=f"m{i}") for i in range(2)]
            qs = [spool.tile([C, free], f16, name=f"q{i}") for i in range(2)]
            yt = yp.tile([C, free], u8, name="yt")
            wt = cp.tile([C, T, C], f16, name="wt")
            pt = ps.tile([C, free], f32, name="pt")
            nc.sync.dma_start(wt[:], w_d[:])

            nbank = free // BANK

            def body():
                # head: the first load lands in 4 column chunks so compute
                # starts after ~1/4 of it; remaining loads are whole tiles,
                # all queued ahead of stores on the SP ring
                hc = free // head_ch
                for j in range(head_ch):
                    sl = slice(j * hc, (j + 1) * hc)
                    nc.sync.dma_start(xts[0][:, sl], x[0][:, sl])
                for j in range(t1_ch):
                    sl = slice(j * (free // t1_ch), (j + 1) * (free // t1_ch))
                    nc.sync.dma_start(xts[1][:, sl], x[1][:, sl])
                for t in range(2, T):
                    nc.sync.dma_start(xts[t][:], x[t])
                q = None
                for t in range(T):
                    u = xts[0] if t == 0 else us[t % 2]
                    last = t == T - 1
                    m = ms[t % 2]
                    # head (t=0,1) and tail (t=7) run column-chunked so the
                    # pipeline fills/drains incrementally
                    if t == 0:
                        nch = head_ch
                    elif last:
                        nch = tail_ch
                    elif t == 1:
                        nch = t1_ch
                    else:
                        nch = 1
                    cw = free // nch
                    for jc in range(nch):
                        sl = slice(jc * cw, (jc + 1) * cw)
                        if t > 0:
                            nc.vector.tensor_tensor(
                                u[:, sl], q[:, sl], xts[t][:, sl], Alu.add
                            )
                        nc.vector.tensor_scalar(
                            m[:, sl], u[:, sl], THRESH, TAU, Alu.is_lt, Alu.mult
                        )
                        if not last:
                            nc.vector.tensor_tensor(
                                qs[t % 2][:, sl], m[:, sl], u[:, sl], Alu.mult
                            )
                        # pack: PSUM bank accumulates 2^(t+2) * m_t
                        for j in range(jc * cw // BANK, (jc + 1) * cw // BANK):
                            bs = slice(j * BANK, (j + 1) * BANK)
                            nc.tensor.matmul(
                                pt[:, bs], wt[:, t, :], m[:, bs],
                                start=(t == 0), stop=last,
                            )
                            if last:
                                # evacuate banks as they finish; DVE is idle
                                # after its last m-chunk, so split the
                                # PSUM->u8 copies between ACT and DVE
                                if j % 2 == 0 or not dve_copies:
                                    nc.scalar.activation(
                                        yt[:, bs], pt[:, bs], Act.Copy,
                                        bias=255.0, scale=-1.0,
                                    )
                                else:
                                    nc.vector.tensor_scalar(
                                        yt[:, bs], pt[:, bs], -1.0, 255.0,
                                        Alu.mult, Alu.add,
                                    )
                        if last:
                            nc.scalar.dma_start(y[:, sl], yt[:, sl])
                    if not last:
                        q = qs[t % 2]

            if loop_n > 1:
                with tc.For_i(0, loop_n):
                    body()
            else:
                body()
    nc.compile()
    return nc


def build_variant(variant, loop_n=1):
    return build_pack(loop_n=loop_n)


def _get_nc():
    key = os.environ.get("LIF_VARIANT", "pack")
    if key not in _nc_cache:
        _nc_cache[key] = build_variant(key)
    return _nc_cache[key]


def host_prep(x):
    """x [B,T,C,H,W] fp32 -> list of per-core [T, C, BPC*HW] fp16 arrays."""
    xs = x.reshape(B, T, C, HW).astype(np.float16)
    return [
        np.ascontiguousarray(
            xs[i * BPC : (i + 1) * BPC].transpose(1, 2, 0, 3).reshape(T, C, FREE)
        )
        for i in range(N_CORES)
    ]


def host_decode(res_list):
    """Per-core packed bytes [C, FREE] -> full fp32 spikes [B,T,C,H,W]."""
    out = np.empty((B, T, C, HW), dtype=np.float32)
    for i, yi in enumerate(res_list):
        bits = np.unpackbits(
            yi.reshape(C, BPC, HW, 1), axis=-1, bitorder="little"
        )  # [C, BPC, HW, 8]
        out[i * BPC : (i + 1) * BPC] = bits.transpose(1, 3, 0, 2)
    return out.reshape(B, T, C, H, W)


def kernel(x):
    global LAST_RESULTS
    from concourse import bass_utils

    assert x.shape == (B, T, C, H, W) and x.dtype == np.float32
    nc = _get_nc()
    in_maps = [{"x": xi} for xi in host_prep(x)]
    res = bass_utils.run_bass_kernel_spmd(
        nc,
        in_maps,
        core_ids=list(range(N_CORES)),
        trace=bool(int(os.environ.get("LIF_TRACE", "0"))),
    )
    LAST_RESULTS = res
    return host_decode([res.results[i]["y"] for i in range(N_CORES)])


# revision 34
# speedup vs baseline: 1.1454x; 1.1374x over previous
"""LIF spike kernel for Trainium2 (Bass/Tile), data-parallel over 8 NeuronCores.

Problem: x [32, 8, 128, 32, 32] fp32 -> spikes [32, 8, 128, 32, 32] fp32
    mem_t = mem_{t-1} * 0.25 + x_t ; spike = (mem >= 0.5) ; mem *= (1 - spike)

Sharding: batch dim (32) split 4-per-core across 8 cores; no cross-core comm.

v3 design (variant "pack", default):
  * All recurrence math in fp16 with DVE ops that have fast perf modes
    (scalar_tensor_tensor has NONE and costs 4.38us on [128,4096] fp32 or
    fp16; tensor_scalar fp16 runs 4x ~1.35us, tensor_tensor fp16 2x ~2.4us):
        u_t = q_{t-1} + x_t            tensor_tensor add   (2x)
        m_t = (u_t < 0.5) * 0.25       tensor_scalar       (4x)  in {0,0.25}
        q_t = m_t * u_t                tensor_tensor mult  (2x)  = tau*reset
  * Spike OUTPUT is bit-packed by the otherwise-idle PE: m_t = 0.25*(1-y_t),
    so PSUM += (2^(t+2) I) @ m_t accumulated over t gives 255 - packed_bits.
    One ACT copy (scale=-1, bias=255) casts PSUM -> uint8; host unpacks bits.
    This removes the per-step ACT Sign op AND cuts store traffic 8x.
  * Host pre-transposes x to [T, C, BPC*HW] fp16 per core so every per-step
    load is one contiguous 1 MiB DMA (128 partitions x 8 KiB lines).
  * Loads ride the SP HWDGE ring; the single store rides the ACT ring.

fp16 rounding (one rounding per step, on u) flips 1772 of 33.5M spikes:
rel err 1.37e-2 < 2e-2, deterministic for the fixed test seed (matches a
numpy emulation of the device op order exactly).

Measured per-iteration (hardware-loop slope, single core): ~52 us, vs ~67 us
for the best scalar_tensor_tensor fp32 formulation. Rejected experiments:
GPSIMD column offload (gpsimd tensor_scalar is 15 ns/col; DVE 2-port ops
block the shared SBUF port), SWDGE accumulate-on-load (CCE fp16 add is
bit-identical, but Q7 descriptor generation contends with DVE 2-port ops:
61 us measured vs 45 us simulated).
"""

import os
import numpy as np

B, T, C, H, W = 32, 8, 128, 32, 32
HW = H * W
N_CORES = 8
BPC = B // N_CORES  # batches per core
FREE = BPC * HW  # 4096 free columns per core
TAU = 0.25
THRESH = 0.5
BANK = 512  # fp32 words per PSUM bank

_nc_cache = {}
LAST_RESULTS = None


# chunk layouts chosen by timeline-sim sweep: finer chunking loses more to
# per-DMA DGE ring entries (~650ns each) than it gains in earlier starts
HEAD_W = (1024, 1024, 1024, 1024)
T1_W = (2048, 2048)
TAIL_W = (2048, 2048)


def build_pack(loop_n=1, head_w=HEAD_W, t1_w=T1_W, tail_w=TAIL_W,
               dve_copies=True):
    import concourse.bacc as bacc
    import concourse.mybir as mybir
    from concourse.tile import TileContext

    f16 = mybir.dt.float16
    f32 = mybir.dt.float32
    u8 = mybir.dt.uint8
    Alu = mybir.AluOpType
    Act = mybir.ActivationFunctionType
    free = FREE

    nc = bacc.Bacc("TRN2", target_bir_lowering=False)
    x = nc.dram_tensor("x", [T, C, free], f16, kind="ExternalInput")
    y = nc.dram_tensor("y", [C, free], u8, kind="ExternalOutput")
    # stationary pack weights: W_t = 2^(t+2) * I so that W_t @ m_t sums to
    # 255 - packed spike bits over the 8 steps. Stored partition-major
    # [C, T, C] so the SBUF tile needs no rearrange.
    ws = np.zeros((C, T, C), dtype=np.float16)
    for t in range(T):
        # m_t in {0, 0.25} for t<7 (weight 2^(t+2) -> contributes 2^t);
        # t=7 uses the {0,1} mask from tt(q6, w7, is_lt) -> weight 2^7
        scale = 2.0 ** (t + 2) if t < T - 1 else 2.0 ** (T - 1)
        ws[np.arange(C), t, np.arange(C)] = np.float16(scale)
    w_d = nc.inline_tensor(ws, "w")

    with TileContext(nc) as tc:
        with (
            tc.tile_pool(name="xp", bufs=1) as xp,
            tc.tile_pool(name="spool", bufs=1) as spool,
            tc.tile_pool(name="yp", bufs=1) as yp,
            tc.tile_pool(name="cp", bufs=1) as cp,
            tc.tile_pool(name="ps", bufs=1, space="PSUM") as ps,
        ):
            xts = [xp.tile([C, free], f16, name=f"x{t}") for t in range(T)]
            us = [spool.tile([C, free], f16, name=f"u{i}") for i in range(2)]
            ms = [spool.tile([C, free], f16, name=f"m{i}") for i in range(2)]
            qs = [spool.tile([C, free], f16, name=f"q{i}") for i in range(2)]
            yt = yp.tile([C, free], u8, name="yt")
            wt = cp.tile([C, T, C], f16, name="wt")
            pt = ps.tile([C, free], f32, name="pt")
            # weight load rides the (idle at head) store ring so it doesn't
            # delay the first x chunks on the SP ring
            nc.scalar.dma_start(wt[:], w_d[:])

            nbank = free // BANK

            def _slices(widths):
                off, out = 0, []
                for w in widths:
                    out.append(slice(off, off + w))
                    off += w
                assert off == free, widths
                return out

            head_sl = _slices(head_w)
            t1_sl = _slices(t1_w)
            tail_sl = _slices(tail_w)

            def body():
                # head: the first loads land in progressively growing column
                # chunks so compute starts ~2us in; remaining loads are whole
                # tiles, all queued ahead of stores on the SP ring
                for sl in head_sl:
                    nc.sync.dma_start(xts[0][:, sl], x[0][:, sl])
                for sl in t1_sl:
                    nc.sync.dma_start(xts[1][:, sl], x[1][:, sl])
                for t in range(2, T):
                    nc.sync.dma_start(xts[t][:], x[t])
                q = None
                for t in range(T):
                    u = xts[0] if t == 0 else us[t % 2]
                    last = t == T - 1
                    m = ms[t % 2]
                    # head (t=0,1) and tail (t=7) run column-chunked so the
                    # pipeline fills/drains incrementally
                    if t == 0:
                        slices = head_sl
                    elif last:
                        slices = tail_sl
                    elif t == 1:
                        slices = t1_sl
                    else:
                        slices = [slice(0, free)]
                    for sl in slices:
                        if last:
                            # x slice 7 holds w7 = 0.5 - x_7 (host-prepped),
                            # so the spike mask needs no u7: m01 = [q6 < w7]
                            nc.vector.tensor_tensor(
                                m[:, sl], q[:, sl], xts[t][:, sl], Alu.is_lt
                            )
                        else:
                            if t > 0:
                                nc.vector.tensor_tensor(
                                    u[:, sl], q[:, sl], xts[t][:, sl], Alu.add
                                )
                            nc.vector.tensor_scalar(
                                m[:, sl], u[:, sl], THRESH, TAU, Alu.is_lt,
                                Alu.mult,
                            )
                        if not last:
                            nc.vector.tensor_tensor(
                                qs[t % 2][:, sl], m[:, sl], u[:, sl], Alu.mult
                            )
                        # pack: PSUM bank accumulates 2^(t+2) * m_t
                        for j in range(sl.start // BANK, sl.stop // BANK):
                            bs = slice(j * BANK, (j + 1) * BANK)
                            nc.tensor.matmul(
                                pt[:, bs], wt[:, t, :], m[:, bs],
                                start=(t == 0), stop=last,
                            )
                            if last:
                                # evacuate banks as they finish; DVE is idle
                                # after its last m-chunk, so split the
                                # PSUM->u8 copies between ACT and DVE
                                if j % 2 == 0 or not dve_copies:
                                    nc.scalar.activation(
                                        yt[:, bs], pt[:, bs], Act.Copy,
                                        bias=255.0, scale=-1.0,
                                    )
                                else:
                                    nc.vector.tensor_scalar(
                                        yt[:, bs], pt[:, bs], -1.0, 255.0,
                                        Alu.mult, Alu.add,
                                    )
                        if last:
                            nc.scalar.dma_start(y[:, sl], yt[:, sl])
                    if not last:
                        q = qs[t % 2]

            if loop_n > 1:
                with tc.For_i(0, loop_n):
                    body()
            else:
                body()
    nc.compile()
    return nc


def build_variant(variant, loop_n=1):
    return build_pack(loop_n=loop_n)


def _get_nc():
    key = os.environ.get("LIF_VARIANT", "pack")
    if key not in _nc_cache:
        _nc_cache[key] = build_variant(key)
    return _nc_cache[key]


def host_prep(x):
    """x [B,T,C,H,W] fp32 -> list of per-core [T, C, BPC*HW] fp16 arrays.
    Slice 7 is replaced by w7 = fp16(0.5 - x_7): the device computes the
    last step's spike mask as [q6 < w7] (equivalent to [q6 + x7 < 0.5]),
    which needs no u7 add. Same shape and traffic."""
    xs = x.reshape(B, T, C, HW).astype(np.float16)
    xs[:, T - 1] = (np.float32(THRESH) - x.reshape(B, T, C, HW)[:, T - 1]).astype(
        np.float16
    )
    return [
        np.ascontiguousarray(
            xs[i * BPC : (i + 1) * BPC].transpose(1, 2, 0, 3).reshape(T, C, FREE)
        )
        for i in range(N_CORES)
    ]


def host_decode(res_list):
    """Per-core packed bytes [C, FREE] -> full fp32 spikes [B,T,C,H,W]."""
    out = np.empty((B, T, C, HW), dtype=np.float32)
    for i, yi in enumerate(res_list):
        bits = np.unpackbits(
            yi.reshape(C, BPC, HW, 1), axis=-1, bitorder="little"
        )  # [C, BPC, HW, 8]
        out[i * BPC : (i + 1) * BPC] = bits.transpose(1, 3, 0, 2)
    return out.reshape(B, T, C, H, W)


def kernel(x):
    global LAST_RESULTS
    from concourse import bass_utils

    assert x.shape == (B, T, C, H, W) and x.dtype == np.float32
    nc = _get_nc()
    in_maps = [{"x": xi} for xi in host_prep(x)]
    res = bass_utils.run_bass_kernel_spmd(
        nc,
        in_maps,
        core_ids=list(range(N_CORES)),
        trace=bool(int(os.environ.get("LIF_TRACE", "0"))),
    )
    LAST_RESULTS = res
    return host_decode([res.results[i]["y"] for i in range(N_CORES)])


# revision 38
# speedup vs baseline: 1.1558x; 1.0091x over previous
"""LIF spike kernel for Trainium2 (Bass/Tile), data-parallel over 8 NeuronCores.

Problem: x [32, 8, 128, 32, 32] fp32 -> spikes [32, 8, 128, 32, 32] fp32
    mem_t = mem_{t-1} * 0.25 + x_t ; spike = (mem >= 0.5) ; mem *= (1 - spike)

Sharding: batch dim (32) split 4-per-core across 8 cores; no cross-core comm.

v3 design (variant "pack", default):
  * All recurrence math in fp16 with DVE ops that have fast perf modes
    (scalar_tensor_tensor has NONE and costs 4.38us on [128,4096] fp32 or
    fp16; tensor_scalar fp16 runs 4x ~1.35us, tensor_tensor fp16 2x ~2.4us):
        u_t = q_{t-1} + x_t            tensor_tensor add   (2x)
        m_t = (u_t < 0.5) * 0.25       tensor_scalar       (4x)  in {0,0.25}
        q_t = m_t * u_t                tensor_tensor mult  (2x)  = tau*reset
  * Spike OUTPUT is bit-packed by the otherwise-idle PE: m_t = 0.25*(1-y_t),
    so PSUM += (2^(t+2) I) @ m_t accumulated over t gives 255 - packed_bits.
    One ACT copy (scale=-1, bias=255) casts PSUM -> uint8; host unpacks bits.
    This removes the per-step ACT Sign op AND cuts store traffic 8x.
  * Host pre-transposes x to [T, C, BPC*HW] fp16 per core so every per-step
    load is one contiguous 1 MiB DMA (128 partitions x 8 KiB lines).
  * Loads ride the SP HWDGE ring; the single store rides the ACT ring.

The last step needs no u7: host sends slice 7 as w7 = fp16(0.5 - x_7) and
the device computes the final spike mask as one tensor_tensor is_lt
(m01 = [q6 < w7], pack weight 2^7), dropping two DVE ops from the tail.

fp16 rounding (one rounding per step, on u) flips 1609 of 33.5M spikes:
rel err 1.31e-2 < 2e-2, deterministic for the fixed test seed (matches a
numpy emulation of the device op order exactly).

Measured per-iteration (hardware-loop slope, single core): ~52 us, vs ~67 us
for the best scalar_tensor_tensor fp32 formulation. Rejected experiments:
GPSIMD column offload (gpsimd tensor_scalar is 15 ns/col; DVE 2-port ops
block the shared SBUF port), SWDGE accumulate-on-load (CCE fp16 add is
bit-identical, but Q7 descriptor generation contends with DVE 2-port ops:
61 us measured vs 45 us simulated).
"""

import os
import numpy as np

B, T, C, H, W = 32, 8, 128, 32, 32
HW = H * W
N_CORES = 8
BPC = B // N_CORES  # batches per core
FREE = BPC * HW  # 4096 free columns per core
TAU = 0.25
THRESH = 0.5
BANK = 512  # fp32 words per PSUM bank

_nc_cache = {}
LAST_RESULTS = None


# chunk layouts chosen by timeline-sim sweep: finer chunking loses more to
# per-DMA DGE ring entries (~650ns each) than it gains in earlier starts
HEAD_W = (1024, 1024, 1024, 1024)
T1_W = (2048, 2048)
TAIL_W = (2048, 2048)


def build_pack(loop_n=1, head_w=HEAD_W, t1_w=T1_W, tail_w=TAIL_W,
               dve_copies=True, gps_cols=0):
    # gps_cols>0 would slice the tensor_tensor ops onto GPSIMD (structurally
    # safe against the shared-port lock), but this toolchain's walrus pass
    # list has no Pool-engine lowering: neuron_isa_check_opcode_on_engine
    # rejects TensorTensor on Pool at codegen. Keep 0.
    import concourse.bacc as bacc
    import concourse.mybir as mybir
    from concourse.tile import TileContext

    f16 = mybir.dt.float16
    f32 = mybir.dt.float32
    u8 = mybir.dt.uint8
    Alu = mybir.AluOpType
    Act = mybir.ActivationFunctionType
    free = FREE

    nc = bacc.Bacc("TRN2", target_bir_lowering=False)
    x = nc.dram_tensor("x", [T, C, free], f16, kind="ExternalInput")
    y = nc.dram_tensor("y", [C, free], u8, kind="ExternalOutput")
    # stationary pack weights: W_t = 2^(t+2) * I so that W_t @ m_t sums to
    # 255 - packed spike bits over the 8 steps. Stored partition-major
    # [C, T, C] so the SBUF tile needs no rearrange.
    ws = np.zeros((C, T, C), dtype=np.float16)
    for t in range(T):
        # m_t in {0, 0.25} for t<7 (weight 2^(t+2) -> contributes 2^t);
        # t=7 uses the {0,1} mask from tt(q6, w7, is_lt) -> weight 2^7
        scale = 2.0 ** (t + 2) if t < T - 1 else 2.0 ** (T - 1)
        ws[np.arange(C), t, np.arange(C)] = np.float16(scale)
    w_d = nc.inline_tensor(ws, "w")

    with TileContext(nc) as tc:
        with (
            tc.tile_pool(name="xp", bufs=1) as xp,
            tc.tile_pool(name="spool", bufs=1) as spool,
            tc.tile_pool(name="yp", bufs=1) as yp,
            tc.tile_pool(name="cp", bufs=1) as cp,
            tc.tile_pool(name="ps", bufs=1, space="PSUM") as ps,
        ):
            xts = [xp.tile([C, free], f16, name=f"x{t}") for t in range(T)]
            us = [spool.tile([C, free], f16, name=f"u{i}") for i in range(2)]
            ms = [spool.tile([C, free], f16, name=f"m{i}") for i in range(2)]
            qs = [spool.tile([C, free], f16, name=f"q{i}") for i in range(2)]
            yt = yp.tile([C, free], u8, name="yt")
            wt = cp.tile([C, T, C], f16, name="wt")
            pt = ps.tile([C, free], f32, name="pt")
            # weight load rides the (idle at head) store ring so it doesn't
            # delay the first x chunks on the SP ring
            nc.scalar.dma_start(wt[:], w_d[:])

            nbank = free // BANK

            def _slices(widths):
                off, out = 0, []
                for w in widths:
                    out.append(slice(off, off + w))
                    off += w
                assert off == free, widths
                return out

            head_sl = _slices(head_w)
            t1_sl = _slices(t1_w)
            tail_sl = _slices(tail_w)

            def body():
                # head: the first loads land in progressively growing column
                # chunks so compute starts ~2us in; remaining loads are whole
                # tiles, all queued ahead of stores on the SP ring
                for sl in head_sl:
                    nc.sync.dma_start(xts[0][:, sl], x[0][:, sl])
                for sl in t1_sl:
                    nc.sync.dma_start(xts[1][:, sl], x[1][:, sl])
                for t in range(2, T):
                    nc.sync.dma_start(xts[t][:], x[t])
                sa = free - gps_cols  # [0,sa) on DVE, [sa,free) on GPSIMD
                gsl = slice(sa, free)

                def pack_mm(t, m, sl, last):
                    for j in range(sl.start // BANK, sl.stop // BANK):
                        bs = slice(j * BANK, (j + 1) * BANK)
                        nc.tensor.matmul(
                            pt[:, bs], wt[:, t, :], m[:, bs],
                            start=(t == 0), stop=last,
                        )
                        if last:
                            # evacuate banks as they finish; DVE is idle
                            # after its last m-chunk, so split the PSUM->u8
                            # copies between ACT and DVE
                            if j % 2 == 0 or not dve_copies:
                                nc.scalar.activation(
                                    yt[:, bs], pt[:, bs], Act.Copy,
                                    bias=255.0, scale=-1.0,
                                )
                            else:
                                nc.vector.tensor_scalar(
                                    yt[:, bs], pt[:, bs], -1.0, 255.0,
                                    Alu.mult, Alu.add,
                                )

                q = None
                for t in range(T - 1):
                    u = xts[0] if t == 0 else us[t % 2]
                    m = ms[t % 2]
                    qn = qs[t % 2]
                    if t == 0:
                        # head: chunked DVE-only fill
                        for sl in head_sl:
                            nc.vector.tensor_scalar(
                                m[:, sl], u[:, sl], THRESH, TAU, Alu.is_lt,
                                Alu.mult,
                            )
                            nc.vector.tensor_tensor(
                                qn[:, sl], m[:, sl], u[:, sl], Alu.mult
                            )
                            pack_mm(t, m, sl, False)
                    else:
                        # interior: the two tensor_tensor ops (1-port on DVE)
                        # give their top gps_cols columns to GPSIMD. The 2-port
                        # m-ts op never overlaps a GPSIMD op: it consumes
                        # GPSIMD's u output and feeds its q input, so the
                        # shared-SBUF-port lock is avoided by construction.
                        u_dve = [slice(0, 2048), slice(2048, sa)] if t == 1 \
                            else [slice(0, sa)]
                        for sl in u_dve:
                            nc.vector.tensor_tensor(
                                u[:, sl], q[:, sl], xts[t][:, sl], Alu.add
                            )
                        if gps_cols:
                            nc.gpsimd.tensor_tensor(
                                u[:, gsl], q[:, gsl], xts[t][:, gsl], Alu.add
                            )
                        nc.vector.tensor_scalar(
                            m[:], u[:], THRESH, TAU, Alu.is_lt, Alu.mult
                        )
                        nc.vector.tensor_tensor(
                            qn[:, :sa], m[:, :sa], u[:, :sa], Alu.mult
                        )
                        if gps_cols:
                            nc.gpsimd.tensor_tensor(
                                qn[:, gsl], m[:, gsl], u[:, gsl], Alu.mult
                            )
                        pack_mm(t, m, slice(0, free), False)
                    q = qn
                # t = 7: x slice 7 holds w7 = 0.5 - x_7 (host-prepped), so the
                # spike mask needs no u7: m01 = [q6 < w7] in {0,1}
                m = ms[(T - 1) % 2]
                for sl in ([slice(0, 2048), slice(2048, sa)] if gps_cols
                           else [slice(0, 2048), slice(2048, free)]):
                    nc.vector.tensor_tensor(
                        m[:, sl], q[:, sl], xts[T - 1][:, sl], Alu.is_lt
                    )
                if gps_cols:
                    nc.gpsimd.tensor_tensor(
                        m[:, gsl], q[:, gsl], xts[T - 1][:, gsl], Alu.is_lt
                    )
                for sl in tail_sl:
                    pack_mm(T - 1, m, sl, True)
                    nc.scalar.dma_start(y[:, sl], yt[:, sl])

            if loop_n > 1:
                with tc.For_i(0, loop_n):
                    body()
            else:
                body()
    nc.compile()
    return nc


def build_variant(variant, loop_n=1):
    return build_pack(loop_n=loop_n)


def _get_nc():
    key = os.environ.get("LIF_VARIANT", "pack")
    if key not in _nc_cache:
        _nc_cache[key] = build_variant(key)
    return _nc_cache[key]


def host_prep(x):
    """x [B,T,C,H,W] fp32 -> list of per-core [T, C, BPC*HW] fp16 arrays.
    Slice 7 is replaced by w7 = fp16(0.5 - x_7): the device computes the
    last step's spike mask as [q6 < w7] (equivalent to [q6 + x7 < 0.5]),
    which needs no u7 add. Same shape and traffic."""
    xs = x.reshape(B, T, C, HW).astype(np.float16)
    xs[:, T - 1] = (np.float32(THRESH) - x.reshape(B, T, C, HW)[:, T - 1]).astype(
        np.float16
    )
    return [
        np.ascontiguousarray(
            xs[i * BPC : (i + 1) * BPC].transpose(1, 2, 0, 3).reshape(T, C, FREE)
        )
        for i in range(N_CORES)
    ]


def host_decode(res_list):
    """Per-core packed bytes [C, FREE] -> full fp32 spikes [B,T,C,H,W]."""
    out = np.empty((B, T, C, HW), dtype=np.float32)
    for i, yi in enumerate(res_list):
        bits = np.unpackbits(
            yi.reshape(C, BPC, HW, 1), axis=-1, bitorder="little"
        )  # [C, BPC, HW, 8]
        out[i * BPC : (i + 1) * BPC] = bits.transpose(1, 3, 0, 2)
    return out.reshape(B, T, C, H, W)


def kernel(x):
    global LAST_RESULTS
    from concourse import bass_utils

    assert x.shape == (B, T, C, H, W) and x.dtype == np.float32
    nc = _get_nc()
    in_maps = [{"x": xi} for xi in host_prep(x)]
    res = bass_utils.run_bass_kernel_spmd(
        nc,
        in_maps,
        core_ids=list(range(N_CORES)),
        trace=bool(int(os.environ.get("LIF_TRACE", "0"))),
    )
    LAST_RESULTS = res
    return host_decode([res.results[i]["y"] for i in range(N_CORES)])


# revision 39
# speedup vs baseline: 1.2099x; 1.0468x over previous
"""LIF spike kernel for Trainium2 (Bass/Tile), data-parallel over 8 NeuronCores.

Problem: x [32, 8, 128, 32, 32] fp32 -> spikes [32, 8, 128, 32, 32] fp32
    mem_t = mem_{t-1} * 0.25 + x_t ; spike = (mem >= 0.5) ; mem *= (1 - spike)

Sharding: batch dim (32) split 4-per-core across 8 cores; no cross-core comm.

v3 design (variant "pack", default):
  * All recurrence math in fp16 with DVE ops that have fast perf modes
    (scalar_tensor_tensor has NONE and costs 4.38us on [128,4096] fp32 or
    fp16; tensor_scalar fp16 runs 4x ~1.35us, tensor_tensor fp16 2x ~2.4us):
        u_t = q_{t-1} + x_t            tensor_tensor add   (2x)
        m_t = (u_t < 0.5) * 0.25       tensor_scalar       (4x)  in {0,0.25}
        q_t = m_t * u_t                tensor_tensor mult  (2x)  = tau*reset
  * Spike OUTPUT is bit-packed by the otherwise-idle PE: m_t = 0.25*(1-y_t),
    so PSUM += (2^(t+2) I) @ m_t accumulated over t gives 255 - packed_bits.
    One ACT copy (scale=-1, bias=255) casts PSUM -> uint8; host unpacks bits.
    This removes the per-step ACT Sign op AND cuts store traffic 8x.
  * Host pre-transposes x to [T, C, BPC*HW] fp16 per core so every per-step
    load is one contiguous 1 MiB DMA (128 partitions x 8 KiB lines).
  * Loads ride the SP HWDGE ring; the single store rides the ACT ring.

The last step needs no u7: host sends slice 7 as w7 = fp16(0.5 - x_7) and
the device computes the final spike mask as one tensor_tensor is_lt
(m01 = [q6 < w7], pack weight 2^7), dropping two DVE ops from the tail.

fp16 rounding (one rounding per step, on u) flips 1609 of 33.5M spikes:
rel err 1.31e-2 < 2e-2, deterministic for the fixed test seed (matches a
numpy emulation of the device op order exactly).

Measured per-iteration (hardware-loop slope, single core): ~52 us, vs ~67 us
for the best scalar_tensor_tensor fp32 formulation. Rejected experiments:
GPSIMD column offload (gpsimd tensor_scalar is 15 ns/col; DVE 2-port ops
block the shared SBUF port), SWDGE accumulate-on-load (CCE fp16 add is
bit-identical, but Q7 descriptor generation contends with DVE 2-port ops:
61 us measured vs 45 us simulated).
"""

import os
import numpy as np

B, T, C, H, W = 32, 8, 128, 32, 32
HW = H * W
N_CORES = 8
BPC = B // N_CORES  # batches per core
FREE = BPC * HW  # 4096 free columns per core
TAU = 0.25
THRESH = 0.5
BANK = 512  # fp32 words per PSUM bank

_nc_cache = {}
LAST_RESULTS = None


# chunk layouts chosen by timeline-sim sweep: finer chunking loses more to
# per-DMA DGE ring entries (~650ns each) than it gains in earlier starts
HEAD_W = (1024, 1024, 1024, 1024)
T1_W = (2048, 2048)
TAIL_W = (2048, 2048)


def build_pack(loop_n=1, head_w=HEAD_W, t1_w=T1_W, tail_w=TAIL_W,
               dve_copies=True, gps_cols=0):
    # gps_cols>0 would slice the tensor_tensor ops onto GPSIMD (structurally
    # safe against the shared-port lock), but this toolchain's walrus pass
    # list has no Pool-engine lowering: neuron_isa_check_opcode_on_engine
    # rejects TensorTensor on Pool at codegen. Keep 0.
    import concourse.bacc as bacc
    import concourse.mybir as mybir
    from concourse.tile import TileContext

    f16 = mybir.dt.float16
    f32 = mybir.dt.float32
    u8 = mybir.dt.uint8
    Alu = mybir.AluOpType
    Act = mybir.ActivationFunctionType
    free = FREE

    nc = bacc.Bacc("TRN2", target_bir_lowering=False)
    x = nc.dram_tensor("x", [T, C, free], f16, kind="ExternalInput")
    y = nc.dram_tensor("y", [C, free], u8, kind="ExternalOutput")
    # stationary pack weights: W_t = 2^(t+2) * I so that W_t @ m_t sums to
    # 255 - packed spike bits over the 8 steps. Stored partition-major
    # [C, T, C] so the SBUF tile needs no rearrange.
    ws = np.zeros((C, T, C), dtype=np.float16)
    for t in range(T):
        # m_t in {0, 0.25} for t<7 (weight 2^(t+2) -> contributes 2^t);
        # t=7 uses the {0,1} mask from tt(q6, w7, is_lt) -> weight 2^7
        scale = 2.0 ** (t + 2) if t < T - 1 else 2.0 ** (T - 1)
        ws[np.arange(C), t, np.arange(C)] = np.float16(scale)
    w_d = nc.inline_tensor(ws, "w")

    with TileContext(nc) as tc:
        with (
            tc.tile_pool(name="xp", bufs=1) as xp,
            tc.tile_pool(name="spool", bufs=1) as spool,
            tc.tile_pool(name="yp", bufs=1) as yp,
            tc.tile_pool(name="cp", bufs=1) as cp,
            tc.tile_pool(name="ps", bufs=1, space="PSUM") as ps,
        ):
            xts = [xp.tile([C, free], f16, name=f"x{t}") for t in range(T)]
            us = [spool.tile([C, free], f16, name=f"u{i}") for i in range(2)]
            ms = [spool.tile([C, free], f16, name=f"m{i}") for i in range(2)]
            qs = [spool.tile([C, free], f16, name=f"q{i}") for i in range(2)]
            yt = yp.tile([C, free], u8, name="yt")
            wt = cp.tile([C, T, C], f16, name="wt")
            pt = ps.tile([C, free], f32, name="pt")
            # weight load rides the (idle at head) store ring so it doesn't
            # delay the first x chunks on the SP ring
            nc.scalar.dma_start(wt[:], w_d[:])

            nbank = free // BANK

            def _slices(widths):
                off, out = 0, []
                for w in widths:
                    out.append(slice(off, off + w))
                    off += w
                assert off == free, widths
                return out

            head_sl = _slices(head_w)
            t1_sl = _slices(t1_w)
            tail_sl = _slices(tail_w)

            def body():
                # head: the first loads land in progressively growing column
                # chunks so compute starts ~2us in; remaining loads are whole
                # tiles, all queued ahead of stores on the SP ring
                for sl in head_sl:
                    nc.sync.dma_start(xts[0][:, sl], x[0][:, sl])
                for sl in t1_sl:
                    nc.sync.dma_start(xts[1][:, sl], x[1][:, sl])
                for t in range(2, T):
                    nc.sync.dma_start(xts[t][:], x[t])
                sa = free - gps_cols  # [0,sa) on DVE, [sa,free) on GPSIMD
                gsl = slice(sa, free)

                def pack_mm(t, m, sl, last):
                    for j in range(sl.start // BANK, sl.stop // BANK):
                        bs = slice(j * BANK, (j + 1) * BANK)
                        nc.tensor.matmul(
                            pt[:, bs], wt[:, t, :], m[:, bs],
                            start=(t == 0), stop=last,
                        )
                        if last:
                            # evacuate banks as they finish; DVE is idle
                            # after its last m-chunk, so split the PSUM->u8
                            # copies between ACT and DVE
                            if j % 2 == 0 or not dve_copies:
                                nc.scalar.activation(
                                    yt[:, bs], pt[:, bs], Act.Copy,
                                    bias=255.0, scale=-1.0,
                                )
                            else:
                                nc.vector.tensor_scalar(
                                    yt[:, bs], pt[:, bs], -1.0, 255.0,
                                    Alu.mult, Alu.add,
                                )

                q = None
                for t in range(T - 1):
                    u = xts[0] if t == 0 else us[t % 2]
                    m = ms[t % 2]
                    qn = qs[t % 2]
                    if t == 0:
                        # head: chunked DVE-only fill
                        for sl in head_sl:
                            nc.vector.tensor_scalar(
                                m[:, sl], u[:, sl], THRESH, TAU, Alu.is_lt,
                                Alu.mult,
                            )
                            nc.vector.tensor_tensor(
                                qn[:, sl], m[:, sl], u[:, sl], Alu.mult
                            )
                            pack_mm(t, m, sl, False)
                    else:
                        # interior: the two tensor_tensor ops (1-port on DVE)
                        # give their top gps_cols columns to GPSIMD. The 2-port
                        # m-ts op never overlaps a GPSIMD op: it consumes
                        # GPSIMD's u output and feeds its q input, so the
                        # shared-SBUF-port lock is avoided by construction.
                        u_dve = [slice(0, 2048), slice(2048, sa)] if t == 1 \
                            else [slice(0, sa)]
                        for sl in u_dve:
                            nc.vector.tensor_tensor(
                                u[:, sl], q[:, sl], xts[t][:, sl], Alu.add
                            )
                        if gps_cols:
                            nc.gpsimd.tensor_tensor(
                                u[:, gsl], q[:, gsl], xts[t][:, gsl], Alu.add
                            )
                        nc.vector.tensor_scalar(
                            m[:], u[:], THRESH, TAU, Alu.is_lt, Alu.mult
                        )
                        nc.vector.tensor_tensor(
                            qn[:, :sa], m[:, :sa], u[:, :sa], Alu.mult
                        )
                        if gps_cols:
                            nc.gpsimd.tensor_tensor(
                                qn[:, gsl], m[:, gsl], u[:, gsl], Alu.mult
                            )
                        pack_mm(t, m, slice(0, free), False)
                    q = qn
                # t = 7: x slice 7 holds w7 = 0.5 - x_7 (host-prepped), so the
                # spike mask needs no u7: m01 = [q6 < w7] in {0,1}
                m = ms[(T - 1) % 2]
                for sl in ([slice(0, 2048), slice(2048, sa)] if gps_cols
                           else [slice(0, 2048), slice(2048, free)]):
                    nc.vector.tensor_tensor(
                        m[:, sl], q[:, sl], xts[T - 1][:, sl], Alu.is_lt
                    )
                if gps_cols:
                    nc.gpsimd.tensor_tensor(
                        m[:, gsl], q[:, gsl], xts[T - 1][:, gsl], Alu.is_lt
                    )
                for sl in tail_sl:
                    pack_mm(T - 1, m, sl, True)
                    # tail stores ride the SP ring: loads drained ~25us ago,
                    # and this keeps DMA dispatch off the ACT sequencer while
                    # it runs the PSUM->u8 copies
                    nc.sync.dma_start(y[:, sl], yt[:, sl])

            if loop_n > 1:
                with tc.For_i(0, loop_n):
                    body()
            else:
                body()
    nc.compile()
    return nc


def build_variant(variant, loop_n=1):
    return build_pack(loop_n=loop_n)


def _get_nc():
    key = os.environ.get("LIF_VARIANT", "pack")
    if key not in _nc_cache:
        _nc_cache[key] = build_variant(key)
    return _nc_cache[key]


def host_prep(x):
    """x [B,T,C,H,W] fp32 -> list of per-core [T, C, BPC*HW] fp16 arrays.
    Slice 7 is replaced by w7 = fp16(0.5 - x_7): the device computes the
    last step's spike mask as [q6 < w7] (equivalent to [q6 + x7 < 0.5]),
    which needs no u7 add. Same shape and traffic."""
    xs = x.reshape(B, T, C, HW).astype(np.float16)
    xs[:, T - 1] = (np.float32(THRESH) - x.reshape(B, T, C, HW)[:, T - 1]).astype(
        np.float16
    )
    return [
        np.ascontiguousarray(
            xs[i * BPC : (i + 1) * BPC].transpose(1, 2, 0, 3).reshape(T, C, FREE)
        )
        for i in range(N_CORES)
    ]


def host_decode(res_list):
    """Per-core packed bytes [C, FREE] -> full fp32 spikes [B,T,C,H,W]."""
    out = np.empty((B, T, C, HW), dtype=np.float32)
    for i, yi in enumerate(res_list):
        bits = np.unpackbits(
            yi.reshape(C, BPC, HW, 1), axis=-1, bitorder="little"
        )  # [C, BPC, HW, 8]
        out[i * BPC : (i + 1) * BPC] = bits.transpose(1, 3, 0, 2)
    return out.reshape(B, T, C, H, W)


def kernel(x):
    global LAST_RESULTS
    from concourse import bass_utils

    assert x.shape == (B, T, C, H, W) and x.dtype == np.float32
    nc = _get_nc()
    in_maps = [{"x": xi} for xi in host_prep(x)]
    res = bass_utils.run_bass_kernel_spmd(
        nc,
        in_maps,
        core_ids=list(range(N_CORES)),
        trace=bool(int(os.environ.get("LIF_TRACE", "0"))),
    )
    LAST_RESULTS = res
    return host_decode([res.results[i]["y"] for i in range(N_CORES)])


# revision 41
# speedup vs baseline: 1.2270x; 1.0142x over previous
"""LIF spike kernel for Trainium2 (Bass/Tile), data-parallel over 8 NeuronCores.

Problem: x [32, 8, 128, 32, 32] fp32 -> spikes [32, 8, 128, 32, 32] fp32
    mem_t = mem_{t-1} * 0.25 + x_t ; spike = (mem >= 0.5) ; mem *= (1 - spike)

Sharding: batch dim (32) split 4-per-core across 8 cores; no cross-core comm.

v3 design (variant "pack", default):
  * All recurrence math in fp16 with DVE ops that have fast perf modes
    (scalar_tensor_tensor has NONE and costs 4.38us on [128,4096] fp32 or
    fp16; tensor_scalar fp16 runs 4x ~1.35us, tensor_tensor fp16 2x ~2.4us):
        u_t = q_{t-1} + x_t            tensor_tensor add   (2x)
        m_t = (u_t < 0.5) * 0.25       tensor_scalar       (4x)  in {0,0.25}
        q_t = m_t * u_t                tensor_tensor mult  (2x)  = tau*reset
  * Spike OUTPUT is bit-packed by the otherwise-idle PE: m_t = 0.25*(1-y_t),
    so PSUM += (2^(t+2) I) @ m_t accumulated over t gives 255 - packed_bits.
    One ACT copy (scale=-1, bias=255) casts PSUM -> uint8; host unpacks bits.
    This removes the per-step ACT Sign op AND cuts store traffic 8x.
  * Host pre-transposes x to [T, C, BPC*HW] fp16 per core so every per-step
    load is one contiguous 1 MiB DMA (128 partitions x 8 KiB lines).
  * Loads ride the SP HWDGE ring; the single store rides the ACT ring.

The last step needs no u7: host sends slice 7 as w7 = fp16(0.5 - x_7) and
the device computes the final spike mask as one tensor_tensor is_lt
(m01 = [q6 < w7], pack weight 2^7), dropping two DVE ops from the tail.

fp16 rounding (one rounding per step, on u) flips 1609 of 33.5M spikes:
rel err 1.31e-2 < 2e-2, deterministic for the fixed test seed (matches a
numpy emulation of the device op order exactly).

Measured per-iteration (hardware-loop slope, single core): ~50-52 us
(device noise +-10%), vs ~67 us for the best scalar_tensor_tensor fp32
formulation. DVE busy is 43 us of that — the structural floor of the
three-op fp16 chain with every op in its fastest available DVE perf mode. Rejected experiments:
GPSIMD column offload (gpsimd tensor_scalar is 15 ns/col; DVE 2-port ops
block the shared SBUF port), SWDGE accumulate-on-load (CCE fp16 add is
bit-identical, but Q7 descriptor generation contends with DVE 2-port ops:
61 us measured vs 45 us simulated).
"""

import os
import numpy as np

B, T, C, H, W = 32, 8, 128, 32, 32
HW = H * W
N_CORES = 8
BPC = B // N_CORES  # batches per core
FREE = BPC * HW  # 4096 free columns per core
TAU = 0.25
THRESH = 0.5
BANK = 512  # fp32 words per PSUM bank

_nc_cache = {}
LAST_RESULTS = None


# chunk layouts chosen by timeline-sim sweep: finer chunking loses more to
# per-DMA DGE ring entries (~650ns each) than it gains in earlier starts
HEAD_W = (1024, 1024, 1024, 1024)
T1_W = (2048, 2048)
TAIL_W = (1536, 1536, 1024)


def build_pack(loop_n=1, head_w=HEAD_W, t1_w=T1_W, tail_w=TAIL_W,
               dve_copies=True, gps_cols=0):
    # gps_cols>0 would slice the tensor_tensor ops onto GPSIMD (structurally
    # safe against the shared-port lock), but this toolchain's walrus pass
    # list has no Pool-engine lowering: neuron_isa_check_opcode_on_engine
    # rejects TensorTensor on Pool at codegen. Keep 0.
    import concourse.bacc as bacc
    import concourse.mybir as mybir
    from concourse.tile import TileContext

    f16 = mybir.dt.float16
    f32 = mybir.dt.float32
    u8 = mybir.dt.uint8
    Alu = mybir.AluOpType
    Act = mybir.ActivationFunctionType
    free = FREE

    nc = bacc.Bacc("TRN2", target_bir_lowering=False)
    x = nc.dram_tensor("x", [T, C, free], f16, kind="ExternalInput")
    y = nc.dram_tensor("y", [C, free], u8, kind="ExternalOutput")
    # stationary pack weights: W_t = 2^(t+2) * I so that W_t @ m_t sums to
    # 255 - packed spike bits over the 8 steps. Stored partition-major
    # [C, T, C] so the SBUF tile needs no rearrange.
    ws = np.zeros((C, T, C), dtype=np.float16)
    for t in range(T):
        # m_t in {0, 0.25} for t<7 (weight 2^(t+2) -> contributes 2^t);
        # t=7 uses the {0,1} mask from tt(q6, w7, is_lt) -> weight 2^7
        scale = 2.0 ** (t + 2) if t < T - 1 else 2.0 ** (T - 1)
        ws[np.arange(C), t, np.arange(C)] = np.float16(scale)
    w_d = nc.inline_tensor(ws, "w")

    with TileContext(nc) as tc:
        with (
            tc.tile_pool(name="xp", bufs=1) as xp,
            tc.tile_pool(name="spool", bufs=1) as spool,
            tc.tile_pool(name="yp", bufs=1) as yp,
            tc.tile_pool(name="cp", bufs=1) as cp,
            tc.tile_pool(name="ps", bufs=1, space="PSUM") as ps,
        ):
            xts = [xp.tile([C, free], f16, name=f"x{t}") for t in range(T)]
            us = [spool.tile([C, free], f16, name=f"u{i}") for i in range(2)]
            ms = [spool.tile([C, free], f16, name=f"m{i}") for i in range(2)]
            qs = [spool.tile([C, free], f16, name=f"q{i}") for i in range(2)]
            yt = yp.tile([C, free], u8, name="yt")
            wt = cp.tile([C, T, C], f16, name="wt")
            pt = ps.tile([C, free], f32, name="pt")
            # weight load rides the (idle at head) store ring so it doesn't
            # delay the first x chunks on the SP ring
            nc.scalar.dma_start(wt[:], w_d[:])

            nbank = free // BANK

            def _slices(widths):
                off, out = 0, []
                for w in widths:
                    out.append(slice(off, off + w))
                    off += w
                assert off == free, widths
                return out

            head_sl = _slices(head_w)
            t1_sl = _slices(t1_w)
            tail_sl = _slices(tail_w)

            def body():
                # head: the first loads land in progressively growing column
                # chunks so compute starts ~2us in; remaining loads are whole
                # tiles, all queued ahead of stores on the SP ring
                for sl in head_sl:
                    nc.sync.dma_start(xts[0][:, sl], x[0][:, sl])
                for sl in t1_sl:
                    nc.sync.dma_start(xts[1][:, sl], x[1][:, sl])
                for t in range(2, T):
                    nc.sync.dma_start(xts[t][:], x[t])
                sa = free - gps_cols  # [0,sa) on DVE, [sa,free) on GPSIMD
                gsl = slice(sa, free)

                def pack_mm(t, m, sl, last):
                    for j in range(sl.start // BANK, sl.stop // BANK):
                        bs = slice(j * BANK, (j + 1) * BANK)
                        nc.tensor.matmul(
                            pt[:, bs], wt[:, t, :], m[:, bs],
                            start=(t == 0), stop=last,
                        )
                        if last:
                            # evacuate banks as they finish; DVE is idle
                            # after its last m-chunk, so split the PSUM->u8
                            # copies between ACT and DVE
                            if j % 2 == 0 or not dve_copies:
                                nc.scalar.activation(
                                    yt[:, bs], pt[:, bs], Act.Copy,
                                    bias=255.0, scale=-1.0,
                                )
                            else:
                                nc.vector.tensor_scalar(
                                    yt[:, bs], pt[:, bs], -1.0, 255.0,
                                    Alu.mult, Alu.add,
                                )

                q = None
                for t in range(T - 1):
                    u = xts[0] if t == 0 else us[t % 2]
                    m = ms[t % 2]
                    qn = qs[t % 2]
                    if t == 0:
                        # head: chunked DVE-only fill
                        for sl in head_sl:
                            nc.vector.tensor_scalar(
                                m[:, sl], u[:, sl], THRESH, TAU, Alu.is_lt,
                                Alu.mult,
                            )
                            nc.vector.tensor_tensor(
                                qn[:, sl], m[:, sl], u[:, sl], Alu.mult
                            )
                            pack_mm(t, m, sl, False)
                    else:
                        # interior: the two tensor_tensor ops (1-port on DVE)
                        # give their top gps_cols columns to GPSIMD. The 2-port
                        # m-ts op never overlaps a GPSIMD op: it consumes
                        # GPSIMD's u output and feeds its q input, so the
                        # shared-SBUF-port lock is avoided by construction.
                        u_dve = [slice(0, 2048), slice(2048, sa)] if t == 1 \
                            else [slice(0, sa)]
                        for sl in u_dve:
                            nc.vector.tensor_tensor(
                                u[:, sl], q[:, sl], xts[t][:, sl], Alu.add
                            )
                        if gps_cols:
                            nc.gpsimd.tensor_tensor(
                                u[:, gsl], q[:, gsl], xts[t][:, gsl], Alu.add
                            )
                        nc.vector.tensor_scalar(
                            m[:], u[:], THRESH, TAU, Alu.is_lt, Alu.mult
                        )
                        nc.vector.tensor_tensor(
                            qn[:, :sa], m[:, :sa], u[:, :sa], Alu.mult
                        )
                        if gps_cols:
                            nc.gpsimd.tensor_tensor(
                                qn[:, gsl], m[:, gsl], u[:, gsl], Alu.mult
                            )
                        pack_mm(t, m, slice(0, free), False)
                    q = qn
                # t = 7: x slice 7 holds w7 = 0.5 - x_7 (host-prepped), so the
                # spike mask needs no u7: m01 = [q6 < w7] in {0,1}
                m = ms[(T - 1) % 2]
                for sl in ([slice(0, 2048), slice(2048, sa)] if gps_cols
                           else [slice(0, 2048), slice(2048, free)]):
                    nc.vector.tensor_tensor(
                        m[:, sl], q[:, sl], xts[T - 1][:, sl], Alu.is_lt
                    )
                if gps_cols:
                    nc.gpsimd.tensor_tensor(
                        m[:, gsl], q[:, gsl], xts[T - 1][:, gsl], Alu.is_lt
                    )
                for sl in tail_sl:
                    pack_mm(T - 1, m, sl, True)
                    # tail stores ride the SP ring: loads drained ~25us ago,
                    # and this keeps DMA dispatch off the ACT sequencer while
                    # it runs the PSUM->u8 copies
                    nc.sync.dma_start(y[:, sl], yt[:, sl])

            if loop_n > 1:
                with tc.For_i(0, loop_n):
                    body()
            else:
                body()
    nc.compile()
    return nc


def build_variant(variant, loop_n=1):
    return build_pack(loop_n=loop_n)


def _get_nc():
    key = os.environ.get("LIF_VARIANT", "pack")
    if key not in _nc_cache:
        _nc_cache[key] = build_variant(key)
    return _nc_cache[key]


def host_prep(x):
    """x [B,T,C,H,W] fp32 -> list of per-core [T, C, BPC*HW] fp16 arrays.
    Slice 7 is replaced by w7 = fp16(0.5 - x_7): the device computes the
    last step's spike mask as [q6 < w7] (equivalent to [q6 + x7 < 0.5]),
    which needs no u7 add. Same shape and traffic."""
    xs = x.reshape(B, T, C, HW).astype(np.float16)
    xs[:, T - 1] = (np.float32(THRESH) - x.reshape(B, T, C, HW)[:, T - 1]).astype(
        np.float16
    )
    return [
        np.ascontiguousarray(
            xs[i * BPC : (i + 1) * BPC].transpose(1, 2, 0, 3).reshape(T, C, FREE)
        )
        for i in range(N_CORES)
    ]


def host_decode(res_list):
    """Per-core packed bytes [C, FREE] -> full fp32 spikes [B,T,C,H,W]."""
    out = np.empty((B, T, C, HW), dtype=np.float32)
    for i, yi in enumerate(res_list):
        bits = np.unpackbits(
            yi.reshape(C, BPC, HW, 1), axis=-1, bitorder="little"
        )  # [C, BPC, HW, 8]
        out[i * BPC : (i + 1) * BPC] = bits.transpose(1, 3, 0, 2)
    return out.reshape(B, T, C, H, W)


def kernel(x):
    global LAST_RESULTS
    from concourse import bass_utils

    assert x.shape == (B, T, C, H, W) and x.dtype == np.float32
    nc = _get_nc()
    in_maps = [{"x": xi} for xi in host_prep(x)]
    res = bass_utils.run_bass_kernel_spmd(
        nc,
        in_maps,
        core_ids=list(range(N_CORES)),
        trace=bool(int(os.environ.get("LIF_TRACE", "0"))),
    )
    LAST_RESULTS = res
    return host_decode([res.results[i]["y"] for i in range(N_CORES)])
